# revision 9
# baseline (speedup 1.0000x reference)
"""MultiHeadDuplexAttention Trainium2 kernel.

Reference computation (per batch item b, fully independent across b):
    Y_new = attend(q_in=X,      kv_in=Y)
    X_new = attend(q_in=Y_new,  kv_in=X)
with attend() = 16-head attention + output projection
    out = (ctx@Wg + bg)*8 + (ctx@Wbeta + bbeta), then @ Wo + bo.

Sharding: pure data-parallel — batch 8 over 8 cores, no collectives.

Host-side algebra (exact up to fp rounding):
  - Wgo = (8*Wg + Wbeta) @ Wo;  bgo = (8*bg + bbeta) @ Wo + bo + bv @ Wgo
    (bv folds through because softmax rows sum to 1)
  - Wq pre-scaled by 1/8 so the 1/sqrt(d_k) is free.

On-chip layout is feature-major (activations transposed, host transposes in/out):
  qT,kvT [D,S] -> QT,KT [D,S] -> scoresT[h] [keys,queries] -> exp ->
  ctxT[h] = V_aug^T-style matmul with a ones column appended to V giving the
  softmax denominator for free in psum row 64 -> normalize via reciprocal +
  PE-broadcast -> out projection (transposed) -> feeds pass 2 directly.

All matmuls run in float32r (single-pass fp32, ~1.5e-4 rel err, 4x the
throughput of strict fp32 on the PE).
"""

import numpy as np
from contextlib import ExitStack

import concourse.bass as bass
from concourse import bacc
import concourse.tile as tile
import concourse.mybir as mybir
from concourse.bass_utils import run_bass_kernel_spmd

F32 = mybir.dt.float32
F32R = mybir.dt.float32r
AF = mybir.ActivationFunctionType
ALU = mybir.AluOpType

B = 8          # batch (== number of cores)
S = 1024       # sequence length
D = 1024       # d_model
H = 16         # heads
DK = 64        # head dim
P = 128        # partitions
NT = D // P    # 8 partition-tiles per [D or S, *] tensor
NCORES = 8
VW = H * (DK + 1)   # 1040: V_aug free width (per head: 64 V cols + 1 ones col)


def _proj_transposed(nc, pmm, wpool, w_dram, rhs_tiles, out_alloc, bias_t, bias_col0):
    """out[mb] (=[128, S]) = W[:, mb-block].T @ rhs  (+ per-partition bias).

    w_dram is [NT, 128, NT*128] host-retiled so tile mb is contiguous:
    w_dram[mb, p, kt*128+f] = W[kt*128+p, mb*128+f].
    """
    for mb in range(NT):
        wt = wpool.tile([P, D], F32R, tag="w", name="w")
        nc.sync.dma_start(wt[:], w_dram[mb].bitcast(F32R))
        ps = pmm.tile([P, S], F32, tag="mm", name="mm")
        for kt in range(NT):
            for qc in range(2):
                nc.tensor.matmul(
                    ps[:, qc * 512:(qc + 1) * 512],
                    wt[:, kt * 128:(kt + 1) * 128],
                    rhs_tiles[kt][:, qc * 512:(qc + 1) * 512],
                    start=(kt == 0), stop=(kt == NT - 1),
                )
        ot = out_alloc(mb)
        nc.vector.tensor_scalar_add(ot[:], ps[:], bias_t[:, bias_col0 + mb:bias_col0 + mb + 1])
        yield mb, ot


def _emit_pass(nc, tc, pools, q_tiles, kv_tiles, w_dram, bias_t, consts,
               qt_pool_tag, ctx_pool_tag, out_writer):
    """One attend() pass. q_tiles/kv_tiles: lists of NT [128,1024] f32r tiles."""
    pmm, pctx, prbc, pE, pW, pK, pV, big_pools = (
        pools["mm"], pools["ctxp"], pools["rbc"], pools["e"], pools["w"],
        pools["k"], pools["v"], pools["big"])
    wq_d, wk_d, wv_d, wgo_d = w_dram
    qt_pool, qt_tag = qt_pool_tag
    ctx_pool, ctx_tag = ctx_pool_tag

    # ---- Phase V: V_aug[st] = (kv @ Wv) with a ones column per head ----
    wv_tiles = []
    for kt in range(NT):
        wvt = pE.tile([P, D], F32R, tag="e", name="e")
        nc.sync.dma_start(wvt[:], wv_d[kt * 128:(kt + 1) * 128, :].bitcast(F32R))
        wv_tiles.append(wvt)
    v_tiles = []
    for st in range(NT):
        ps = pmm.tile([P, D], F32, tag="mm", name="mm")
        for kt in range(NT):
            for dc in range(2):
                nc.tensor.matmul(
                    ps[:, dc * 512:(dc + 1) * 512],
                    kv_tiles[kt][:, st * 128:(st + 1) * 128],
                    wv_tiles[kt][:, dc * 512:(dc + 1) * 512],
                    start=(kt == 0), stop=(kt == NT - 1),
                )
        vt = pV.tile([P, VW], F32R, tag=f"v{st}", name=f"v{st}")
        vr = vt[:].rearrange("p (h c) -> p h c", c=DK + 1)
        nc.vector.tensor_copy(vr[:, :, DK:DK + 1], consts["col128"])
        for dc in range(2):
            nc.vector.tensor_copy(
                vr[:, dc * 8:(dc + 1) * 8, 0:DK],
                ps[:, dc * 512:(dc + 1) * 512].rearrange("p (h c) -> p h c", c=DK),
            )
        v_tiles.append(vt)

    # ---- Phase K/Q: KT[mb], QT[mb] ----
    kt_tiles = [None] * NT
    for mb, t in _proj_transposed(nc, pmm, pW, wk_d, kv_tiles,
                                  lambda mb: pK.tile([P, S], F32R, tag=f"k{mb}", name=f"k{mb}"),
                                  bias_t, 8):
        kt_tiles[mb] = t
    qt_tiles = [None] * NT
    for mb, t in _proj_transposed(nc, pmm, pW, wq_d, q_tiles,
                                  lambda mb: qt_pool.tile([P, S], F32R, tag=f"{qt_tag}{mb}", name=f"qt{mb}"),
                                  bias_t, 0):
        qt_tiles[mb] = t

    # ---- Attention ----
    ctx_tiles = [None] * NT
    for h in range(H):
        tp, poff = h // 2, (h % 2) * DK
        if ctx_tiles[tp] is None:
            ctx_tiles[tp] = ctx_pool.tile([P, S], F32R, tag=f"{ctx_tag}{tp}", name=f"ctx{tp}")
        e_tiles = []
        for kt in range(NT):
            ps = pmm.tile([P, S], F32, tag="mm", name="mm")
            for qc in range(2):
                nc.tensor.matmul(
                    ps[:, qc * 512:(qc + 1) * 512],
                    kt_tiles[tp][poff:poff + DK, kt * 128:(kt + 1) * 128],
                    qt_tiles[tp][poff:poff + DK, qc * 512:(qc + 1) * 512],
                    start=True, stop=True,
                )
            et = pE.tile([P, S], F32R, tag="e", name="e")
            nc.scalar.activation(et[:], ps[:], AF.Exp)
            e_tiles.append(et)
        for qc in range(2):
            cps = pctx.tile([DK + 1, 512], F32, tag="ctxp", name="ctxp")
            for kt in range(NT):
                nc.tensor.matmul(
                    cps[:],
                    v_tiles[kt][:, h * (DK + 1):(h + 1) * (DK + 1)],
                    e_tiles[kt][:, qc * 512:(qc + 1) * 512],
                    start=(kt == 0), stop=(kt == NT - 1),
                )
            r = pools["r"].tile([1, 512], F32R, tag="r", name="r")
            nc.vector.reciprocal(r[:], cps[DK:DK + 1, :])
            rb = prbc.tile([DK, 512], F32, tag="rbc", name="rbc")
            nc.tensor.matmul(rb[:], consts["ones64"], r[:], start=True, stop=True)
            rbs = pools["rbs"].tile([DK, 512], F32, tag="rbs", name="rbs")
            nc.scalar.copy(rbs[:], rb[:])
            nc.vector.tensor_tensor(
                ctx_tiles[tp][poff:poff + DK, qc * 512:(qc + 1) * 512],
                cps[0:DK, :], rbs[:], ALU.mult,
            )

    # ---- Output projection (transposed) ----
    for mb, t in _proj_transposed(nc, pmm, pW, wgo_d, ctx_tiles,
                                  lambda mb: out_writer[0](mb), bias_t, 16):
        out_writer[1](mb, t)
    return


def build():
    nc = bacc.Bacc(None)
    xT = nc.declare_dram_parameter("xT", [D, S], F32, isOutput=False)
    yT = nc.declare_dram_parameter("yT", [D, S], F32, isOutput=False)
    wq = nc.declare_dram_parameter("wq", [NT, P, D], F32, isOutput=False)
    wk = nc.declare_dram_parameter("wk", [NT, P, D], F32, isOutput=False)
    wv = nc.declare_dram_parameter("wv", [D, D], F32, isOutput=False)
    wgo = nc.declare_dram_parameter("wgo", [NT, P, D], F32, isOutput=False)
    bias = nc.declare_dram_parameter("bias", [P, 24], F32, isOutput=False)
    ynewT = nc.declare_dram_parameter("ynewT", [D, S], F32, isOutput=True)
    xnewT = nc.declare_dram_parameter("xnewT", [D, S], F32, isOutput=True)

    with nc.allow_low_precision("fp32r matmul pipeline by design"), \
         tile.TileContext(nc) as tc, ExitStack() as ctx:
        pA = ctx.enter_context(tc.tile_pool(name="pA", bufs=1))
        pB = ctx.enter_context(tc.tile_pool(name="pB", bufs=1))
        pK = ctx.enter_context(tc.tile_pool(name="pK", bufs=1))
        pV = ctx.enter_context(tc.tile_pool(name="pV", bufs=1))
        pE = ctx.enter_context(tc.tile_pool(name="pE", bufs=8))
        pW = ctx.enter_context(tc.tile_pool(name="pW", bufs=3))
        pR = ctx.enter_context(tc.tile_pool(name="pR", bufs=2))
        pOut = ctx.enter_context(tc.tile_pool(name="pOut", bufs=2))
        pRbs = ctx.enter_context(tc.tile_pool(name="pRbs", bufs=2))
        pMisc = ctx.enter_context(tc.tile_pool(name="pMisc", bufs=1))
        pmm = ctx.enter_context(tc.tile_pool(name="pmm", bufs=2, space="PSUM"))
        pctx = ctx.enter_context(tc.tile_pool(name="pctx", bufs=2, space="PSUM"))
        prbc = ctx.enter_context(tc.tile_pool(name="prbc", bufs=2, space="PSUM"))

        pools = dict(mm=pmm, ctxp=pctx, rbc=prbc, e=pE, w=pW, k=pK, v=pV,
                     r=pR, rbs=pRbs, big=(pA, pB))

        bias_t = pMisc.tile([P, 24], F32, tag="bias", name="bias")
        nc.sync.dma_start(bias_t[:], bias[:])
        ones_f = pMisc.tile([P, DK], F32, tag="onesf", name="onesf")
        nc.vector.memset(ones_f[:], 1.0)
        ones_t = pMisc.tile([1, DK], F32R, tag="ones", name="ones")
        nc.vector.tensor_copy(ones_t[:], ones_f[0:1, :])
        consts = dict(ones64=ones_t[:],
                      col128=ones_f[:, 0:16].unsqueeze(2))

        # load inputs: YT -> A, XT -> B
        a_tiles = []
        b_tiles = []
        for i in range(NT):
            at = pA.tile([P, S], F32R, tag=f"a{i}", name=f"a{i}")
            nc.sync.dma_start(at[:], yT[i * 128:(i + 1) * 128, :].bitcast(F32R))
            a_tiles.append(at)
            bt = pB.tile([P, S], F32R, tag=f"b{i}", name=f"b{i}")
            nc.sync.dma_start(bt[:], xT[i * 128:(i + 1) * 128, :].bitcast(F32R))
            b_tiles.append(bt)

        w_dram = (wq, wk, wv, wgo)

        # ---- pass 1: q = X (B), kv = Y (A); QT1->A, ctx1->B, Ynew->A ----
        ynew_tiles = [None] * NT

        def p1_alloc(mb):
            t = pA.tile([P, S], F32R, tag=f"a{mb}", name=f"yn{mb}")
            ynew_tiles[mb] = t
            return t

        def p1_write(mb, t):
            nc.sync.dma_start(ynewT[mb * 128:(mb + 1) * 128, :].bitcast(F32R), t[:])

        _emit_pass(nc, tc, pools, b_tiles, a_tiles, w_dram, bias_t, consts,
                   qt_pool_tag=(pA, "a"), ctx_pool_tag=(pB, "b"),
                   out_writer=(p1_alloc, p1_write))

        # ---- pass 2: q = Y_new (A), kv = X reloaded (B); QT2->B, ctx2->A ----
        b2_tiles = []
        for i in range(NT):
            bt = pB.tile([P, S], F32R, tag=f"b{i}", name=f"b{i}")
            nc.sync.dma_start(bt[:], xT[i * 128:(i + 1) * 128, :].bitcast(F32R))
            b2_tiles.append(bt)

        def p2_alloc(mb):
            return pOut.tile([P, S], F32, tag="out", name="out")

        def p2_write(mb, t):
            nc.sync.dma_start(xnewT[mb * 128:(mb + 1) * 128, :], t[:])

        _emit_pass(nc, tc, pools, ynew_tiles, b2_tiles, w_dram, bias_t, consts,
                   qt_pool_tag=(pB, "b"), ctx_pool_tag=(pA, "a"),
                   out_writer=(p2_alloc, p2_write))

    nc.finalize()
    return nc


def _retile_w(w):
    # [mb, p, kt*128+f] = w[kt*128+p, mb*128+f]
    return np.ascontiguousarray(
        w.reshape(NT, P, NT, P).transpose(2, 1, 0, 3).reshape(NT, P, D))


def _prep_host(inputs):
    f64 = np.float64
    Wq = np.asarray(inputs["Wq"], f64); bq = np.asarray(inputs["bq"], f64)
    Wk = np.asarray(inputs["Wk"], f64); bk = np.asarray(inputs["bk"], f64)
    Wv = np.asarray(inputs["Wv"], f64); bv = np.asarray(inputs["bv"], f64)
    Wg = np.asarray(inputs["Wg"], f64); bg = np.asarray(inputs["bg"], f64)
    Wb = np.asarray(inputs["Wbeta"], f64); bb = np.asarray(inputs["bbeta"], f64)
    Wo = np.asarray(inputs["Wo"], f64); bo = np.asarray(inputs["bo"], f64)

    sc = np.sqrt(np.float64(DK))          # == 8
    Wgo = (sc * Wg + Wb) @ Wo
    bgo = (sc * bg + bb) @ Wo + bo + bv @ Wgo

    wq_t = _retile_w((Wq / 8.0).astype(np.float32))
    wk_t = _retile_w(Wk.astype(np.float32))
    wgo_t = _retile_w(Wgo.astype(np.float32))
    wv_n = np.ascontiguousarray(Wv.astype(np.float32))

    bias = np.zeros((P, 24), np.float32)
    bias[:, 0:8] = (bq / 8.0).astype(np.float32).reshape(NT, P).T
    bias[:, 8:16] = bk.astype(np.float32).reshape(NT, P).T
    bias[:, 16:24] = bgo.astype(np.float32).reshape(NT, P).T
    return wq_t, wk_t, wv_n, wgo_t, bias


_NC_CACHE = [None]


def kernel(**inputs):
    X = np.asarray(inputs["X"], np.float32)
    Y = np.asarray(inputs["Y"], np.float32)
    wq_t, wk_t, wv_n, wgo_t, bias = _prep_host(inputs)

    if _NC_CACHE[0] is None:
        _NC_CACHE[0] = build()
    nc = _NC_CACHE[0]

    in_maps = []
    for b in range(B):
        in_maps.append(dict(
            xT=np.ascontiguousarray(X[b].T),
            yT=np.ascontiguousarray(Y[b].T),
            wq=wq_t, wk=wk_t, wv=wv_n, wgo=wgo_t, bias=bias,
        ))
    res = run_bass_kernel_spmd(nc, in_maps, core_ids=list(range(NCORES)))

    X_new = np.empty((B, S, D), np.float32)
    Y_new = np.empty((B, S, D), np.float32)
    for b in range(B):
        X_new[b] = res.results[b]["xnewT"].T
        Y_new[b] = res.results[b]["ynewT"].T
    return (X_new, Y_new)


# revision 12
# speedup vs baseline: 1.5918x; 1.5918x over previous
"""MultiHeadDuplexAttention Trainium2 kernel.

Reference computation (per batch item b, fully independent across b):
    Y_new = attend(q_in=X,      kv_in=Y)
    X_new = attend(q_in=Y_new,  kv_in=X)
with attend() = 16-head attention + output projection
    out = (ctx@Wg + bg)*8 + (ctx@Wbeta + bbeta), then @ Wo + bo.

Sharding: pure data-parallel — batch 8 over 8 cores, no collectives.

Host-side algebra (exact up to fp rounding):
  - Wgo = (8*Wg + Wbeta) @ Wo;  bgo = (8*bg + bbeta) @ Wo + bo + bv @ Wgo
    (bv folds through because softmax rows sum to 1)
  - Wq pre-scaled by 1/8 so the 1/sqrt(d_k) is free.

On-chip layout is feature-major (activations transposed, host transposes in/out):
  qT,kvT [D,S] -> QT,KT [D,S] -> scoresT[h] [keys,queries] -> exp ->
  ctxT[h] = V_aug^T-style matmul with a ones column appended to V giving the
  softmax denominator for free in psum row 64 -> normalize via reciprocal +
  PE-broadcast -> out projection (transposed) -> feeds pass 2 directly.

All matmuls run in float32r (single-pass fp32, ~1.5e-4 rel err, 4x the
throughput of strict fp32 on the PE).
"""

import numpy as np
from contextlib import ExitStack

import concourse.bass as bass
from concourse import bacc
import concourse.tile as tile
import concourse.mybir as mybir
from concourse.bass_utils import run_bass_kernel_spmd

F32 = mybir.dt.float32
F32R = mybir.dt.float32r
AF = mybir.ActivationFunctionType
ALU = mybir.AluOpType

B = 8          # batch (== number of cores)
S = 1024       # sequence length
D = 1024       # d_model
H = 16         # heads
DK = 64        # head dim
P = 128        # partitions
NT = D // P    # 8 partition-tiles per [D or S, *] tensor
NCORES = 8
VW = H * (DK + 1)   # 1040: V_aug free width (per head: 64 V cols + 1 ones col)


def _proj_transposed(nc, pmm, wpool, w_dram, rhs_tiles, out_alloc, bias_t, bias_col0):
    """out[mb] (=[128, S]) = W[:, mb-block].T @ rhs  (+ per-partition bias).

    w_dram is [NT, 128, NT*128] host-retiled so tile mb is contiguous:
    w_dram[mb, p, kt*128+f] = W[kt*128+p, mb*128+f].
    """
    for mb in range(NT):
        wt = wpool.tile([P, D], F32R, tag="w", name="w")
        nc.sync.dma_start(wt[:], w_dram[mb].bitcast(F32R))
        ps = pmm.tile([P, S], F32, tag="mm", name="mm")
        for kt in range(NT):
            for qc in range(2):
                nc.tensor.matmul(
                    ps[:, qc * 512:(qc + 1) * 512],
                    wt[:, kt * 128:(kt + 1) * 128],
                    rhs_tiles[kt][:, qc * 512:(qc + 1) * 512],
                    start=(kt == 0), stop=(kt == NT - 1),
                )
        ot = out_alloc(mb)
        nc.vector.tensor_scalar_add(ot[:], ps[:], bias_t[:, bias_col0 + mb:bias_col0 + mb + 1])
        yield mb, ot


def _emit_pass(nc, tc, pools, q_tiles, kv_tiles, w_dram, bias_t, consts,
               qt_pool_tag, ctx_pool_tag, out_writer):
    """One attend() pass. q_tiles/kv_tiles: lists of NT [128,1024] f32r tiles."""
    pmm, pctx, pE, pW, pK, pV = (
        pools["mm"], pools["ctxp"], pools["e"], pools["w"],
        pools["k"], pools["v"])
    wq_d, wk_d, wv_d, wgo_d = w_dram
    qt_pool, qt_tag = qt_pool_tag
    ctx_pool, ctx_tag = ctx_pool_tag

    # ---- Phase V: V_aug[st] = (kv @ Wv) with a ones column per head ----
    wv_tiles = []
    for kt in range(NT):
        wvt = pE.tile([P, D], F32R, tag="e", name="e")
        nc.sync.dma_start(wvt[:], wv_d[kt * 128:(kt + 1) * 128, :].bitcast(F32R))
        wv_tiles.append(wvt)
    v_tiles = []
    for st in range(NT):
        ps = pmm.tile([P, D], F32, tag="mm", name="mm")
        for kt in range(NT):
            for dc in range(2):
                nc.tensor.matmul(
                    ps[:, dc * 512:(dc + 1) * 512],
                    kv_tiles[kt][:, st * 128:(st + 1) * 128],
                    wv_tiles[kt][:, dc * 512:(dc + 1) * 512],
                    start=(kt == 0), stop=(kt == NT - 1),
                )
        vt = pV.tile([P, VW], F32R, tag=f"v{st}", name=f"v{st}")
        vr = vt[:].rearrange("p (h c) -> p h c", c=DK + 1)
        nc.vector.tensor_copy(vr[:, :, DK:DK + 1], consts["col128"])
        for dc in range(2):
            nc.vector.tensor_copy(
                vr[:, dc * 8:(dc + 1) * 8, 0:DK],
                ps[:, dc * 512:(dc + 1) * 512].rearrange("p (h c) -> p h c", c=DK),
            )
        v_tiles.append(vt)

    # ---- Phase K/Q: KT[mb], QT[mb] ----
    kt_tiles = [None] * NT
    for mb, t in _proj_transposed(nc, pmm, pW, wk_d, kv_tiles,
                                  lambda mb: pK.tile([P, S], F32R, tag=f"k{mb}", name=f"k{mb}"),
                                  bias_t, 8):
        kt_tiles[mb] = t
    qt_tiles = [None] * NT
    for mb, t in _proj_transposed(nc, pmm, pW, wq_d, q_tiles,
                                  lambda mb: qt_pool.tile([P, S], F32R, tag=f"{qt_tag}{mb}", name=f"qt{mb}"),
                                  bias_t, 0):
        qt_tiles[mb] = t

    # ---- Attention ----
    ctx_tiles = [None] * NT
    for h in range(H):
        tp, poff = h // 2, (h % 2) * DK
        if ctx_tiles[tp] is None:
            ctx_tiles[tp] = ctx_pool.tile([P, S], F32R, tag=f"{ctx_tag}{tp}", name=f"ctx{tp}")
        e_tiles = []
        for kt in range(NT):
            ps = pmm.tile([P, S], F32, tag="mm", name="mm")
            for qc in range(2):
                nc.tensor.matmul(
                    ps[:, qc * 512:(qc + 1) * 512],
                    kt_tiles[tp][poff:poff + DK, kt * 128:(kt + 1) * 128],
                    qt_tiles[tp][poff:poff + DK, qc * 512:(qc + 1) * 512],
                    start=True, stop=True,
                )
            et = pE.tile([P, S], F32R, tag="e", name="e")
            nc.scalar.activation(et[:], ps[:], AF.Exp)
            e_tiles.append(et)
        for qc in range(2):
            cps = pctx.tile([DK + 1, 512], F32, tag="ctxp", name="ctxp")
            for kt in range(NT):
                nc.tensor.matmul(
                    cps[:],
                    v_tiles[kt][:, h * (DK + 1):(h + 1) * (DK + 1)],
                    e_tiles[kt][:, qc * 512:(qc + 1) * 512],
                    start=(kt == 0), stop=(kt == NT - 1),
                )
            # raw-evict immediately so the psum bank frees fast (keeps PE hot);
            # the whole normalize chain below is DVE+GPSIMD only.
            craw = pools["craw"].tile([DK + 1, 512], F32, tag="craw", name="craw")
            nc.vector.tensor_copy(craw[:], cps[:])
            # custom DVE ops ignore input base_partition: copy denom row to
            # a partition-0 tile (native copy handles the offset) first.
            dr = pools["r"].tile([1, 512], F32, tag="dr", name="dr")
            nc.vector.tensor_copy(dr[:], craw[DK:DK + 1, :])
            r = pools["r"].tile([1, 512], F32, tag="r", name="r")
            nc.vector.reciprocal_approx_fast(r[:], dr[:])
            rbs = pools["rbs"].tile([DK, 512], F32, tag="rbs", name="rbs")
            nc.gpsimd.partition_broadcast(rbs[:], r[:])
            nc.vector.tensor_tensor(
                ctx_tiles[tp][poff:poff + DK, qc * 512:(qc + 1) * 512],
                craw[0:DK, :], rbs[:], ALU.mult,
            )

    # ---- Output projection (transposed) ----
    for mb, t in _proj_transposed(nc, pmm, pW, wgo_d, ctx_tiles,
                                  lambda mb: out_writer[0](mb), bias_t, 16):
        out_writer[1](mb, t)
    return


def build():
    nc = bacc.Bacc(None)
    xT = nc.declare_dram_parameter("xT", [D, S], F32, isOutput=False)
    yT = nc.declare_dram_parameter("yT", [D, S], F32, isOutput=False)
    wq = nc.declare_dram_parameter("wq", [NT, P, D], F32, isOutput=False)
    wk = nc.declare_dram_parameter("wk", [NT, P, D], F32, isOutput=False)
    wv = nc.declare_dram_parameter("wv", [D, D], F32, isOutput=False)
    wgo = nc.declare_dram_parameter("wgo", [NT, P, D], F32, isOutput=False)
    bias = nc.declare_dram_parameter("bias", [P, 24], F32, isOutput=False)
    ynewT = nc.declare_dram_parameter("ynewT", [D, S], F32, isOutput=True)
    xnewT = nc.declare_dram_parameter("xnewT", [D, S], F32, isOutput=True)

    with nc.allow_low_precision("fp32r matmul pipeline by design"), \
         tile.TileContext(nc) as tc, ExitStack() as ctx:
        pA = ctx.enter_context(tc.tile_pool(name="pA", bufs=1))
        pB = ctx.enter_context(tc.tile_pool(name="pB", bufs=1))
        pK = ctx.enter_context(tc.tile_pool(name="pK", bufs=1))
        pV = ctx.enter_context(tc.tile_pool(name="pV", bufs=1))
        pE = ctx.enter_context(tc.tile_pool(name="pE", bufs=8))
        pW = ctx.enter_context(tc.tile_pool(name="pW", bufs=3))
        pR = ctx.enter_context(tc.tile_pool(name="pR", bufs=2))
        pOut = ctx.enter_context(tc.tile_pool(name="pOut", bufs=2))
        pRbs = ctx.enter_context(tc.tile_pool(name="pRbs", bufs=3))
        pCraw = ctx.enter_context(tc.tile_pool(name="pCraw", bufs=4))
        pMisc = ctx.enter_context(tc.tile_pool(name="pMisc", bufs=1))
        pmm = ctx.enter_context(tc.tile_pool(name="pmm", bufs=2, space="PSUM"))
        pctx = ctx.enter_context(tc.tile_pool(name="pctx", bufs=4, space="PSUM"))

        pools = dict(mm=pmm, ctxp=pctx, e=pE, w=pW, k=pK, v=pV,
                     r=pR, rbs=pRbs, craw=pCraw, big=(pA, pB))

        bias_t = pMisc.tile([P, 24], F32, tag="bias", name="bias")
        nc.sync.dma_start(bias_t[:], bias[:])
        ones_f = pMisc.tile([P, DK], F32, tag="onesf", name="onesf")
        nc.vector.memset(ones_f[:], 1.0)
        consts = dict(col128=ones_f[:, 0:16].unsqueeze(2))

        # load inputs: YT -> A, XT -> B
        a_tiles = []
        b_tiles = []
        for i in range(NT):
            at = pA.tile([P, S], F32R, tag=f"a{i}", name=f"a{i}")
            nc.sync.dma_start(at[:], yT[i * 128:(i + 1) * 128, :].bitcast(F32R))
            a_tiles.append(at)
            bt = pB.tile([P, S], F32R, tag=f"b{i}", name=f"b{i}")
            nc.sync.dma_start(bt[:], xT[i * 128:(i + 1) * 128, :].bitcast(F32R))
            b_tiles.append(bt)

        w_dram = (wq, wk, wv, wgo)

        # ---- pass 1: q = X (B), kv = Y (A); QT1->A, ctx1->B, Ynew->A ----
        ynew_tiles = [None] * NT

        def p1_alloc(mb):
            t = pA.tile([P, S], F32R, tag=f"a{mb}", name=f"yn{mb}")
            ynew_tiles[mb] = t
            return t

        def p1_write(mb, t):
            nc.sync.dma_start(ynewT[mb * 128:(mb + 1) * 128, :].bitcast(F32R), t[:])

        _emit_pass(nc, tc, pools, b_tiles, a_tiles, w_dram, bias_t, consts,
                   qt_pool_tag=(pA, "a"), ctx_pool_tag=(pB, "b"),
                   out_writer=(p1_alloc, p1_write))

        # ---- pass 2: q = Y_new (A), kv = X reloaded (B); QT2->B, ctx2->A ----
        b2_tiles = []
        for i in range(NT):
            bt = pB.tile([P, S], F32R, tag=f"b{i}", name=f"b{i}")
            nc.sync.dma_start(bt[:], xT[i * 128:(i + 1) * 128, :].bitcast(F32R))
            b2_tiles.append(bt)

        def p2_alloc(mb):
            return pOut.tile([P, S], F32, tag="out", name="out")

        def p2_write(mb, t):
            nc.sync.dma_start(xnewT[mb * 128:(mb + 1) * 128, :], t[:])

        _emit_pass(nc, tc, pools, ynew_tiles, b2_tiles, w_dram, bias_t, consts,
                   qt_pool_tag=(pB, "b"), ctx_pool_tag=(pA, "a"),
                   out_writer=(p2_alloc, p2_write))

    nc.finalize()
    return nc


def _retile_w(w):
    # [mb, p, kt*128+f] = w[kt*128+p, mb*128+f]
    return np.ascontiguousarray(
        w.reshape(NT, P, NT, P).transpose(2, 1, 0, 3).reshape(NT, P, D))


def _prep_host(inputs):
    f64 = np.float64
    Wq = np.asarray(inputs["Wq"], f64); bq = np.asarray(inputs["bq"], f64)
    Wk = np.asarray(inputs["Wk"], f64); bk = np.asarray(inputs["bk"], f64)
    Wv = np.asarray(inputs["Wv"], f64); bv = np.asarray(inputs["bv"], f64)
    Wg = np.asarray(inputs["Wg"], f64); bg = np.asarray(inputs["bg"], f64)
    Wb = np.asarray(inputs["Wbeta"], f64); bb = np.asarray(inputs["bbeta"], f64)
    Wo = np.asarray(inputs["Wo"], f64); bo = np.asarray(inputs["bo"], f64)

    sc = np.sqrt(np.float64(DK))          # == 8
    Wgo = (sc * Wg + Wb) @ Wo
    bgo = (sc * bg + bb) @ Wo + bo + bv @ Wgo

    wq_t = _retile_w((Wq / 8.0).astype(np.float32))
    wk_t = _retile_w(Wk.astype(np.float32))
    wgo_t = _retile_w(Wgo.astype(np.float32))
    wv_n = np.ascontiguousarray(Wv.astype(np.float32))

    bias = np.zeros((P, 24), np.float32)
    bias[:, 0:8] = (bq / 8.0).astype(np.float32).reshape(NT, P).T
    bias[:, 8:16] = bk.astype(np.float32).reshape(NT, P).T
    bias[:, 16:24] = bgo.astype(np.float32).reshape(NT, P).T
    return wq_t, wk_t, wv_n, wgo_t, bias


_NC_CACHE = [None]


def kernel(**inputs):
    X = np.asarray(inputs["X"], np.float32)
    Y = np.asarray(inputs["Y"], np.float32)
    wq_t, wk_t, wv_n, wgo_t, bias = _prep_host(inputs)

    if _NC_CACHE[0] is None:
        _NC_CACHE[0] = build()
    nc = _NC_CACHE[0]

    in_maps = []
    for b in range(B):
        in_maps.append(dict(
            xT=np.ascontiguousarray(X[b].T),
            yT=np.ascontiguousarray(Y[b].T),
            wq=wq_t, wk=wk_t, wv=wv_n, wgo=wgo_t, bias=bias,
        ))
    res = run_bass_kernel_spmd(nc, in_maps, core_ids=list(range(NCORES)))

    X_new = np.empty((B, S, D), np.float32)
    Y_new = np.empty((B, S, D), np.float32)
    for b in range(B):
        X_new[b] = res.results[b]["xnewT"].T
        Y_new[b] = res.results[b]["ynewT"].T
    return (X_new, Y_new)
